# revision 6
# baseline (speedup 1.0000x reference)
"""CAM (channel attention) kernel V2 for Trainium2, data-parallel over batch.

Per sample:
    v = x.reshape(C, N); energy = v @ v.T
    att = softmax(rowmax(energy) - energy, axis=-1)  [= exp(rowmin-E)/rowsum]
    out = gamma * (att @ v) + x

Key structure changes vs V1:
  - all PE transposes use a BF16 identity: the transpose cycle rate is keyed
    on the moving operand (the identity), so bf16 gives 1.0 cyc/row vs 1.5
    for the fp32r identity (numerically exact - identity is 0/1)
  - energy operands are FP16 (vt16 built by the PSUM->SBUF copy conversion):
    fp16 matmuls run at 1 cyc/row at ANY free width, so the triangular row
    blocks shrink to widths [512,384,256,128] (fp32r needs >=256). fp16
    operand rounding (2^-11) adds ~1.5e-2 logit noise - measured end-to-end
    error stays at the baseline 3.4e-3 (bf16 output rounding dominates)
  - the attention-transpose phase is GONE: since E is symmetric, the
    transposed unnormalized attention is attT[d,c] = exp(min[c] - E[d,c]),
    computed directly on the stored energy tiles with a free-axis bias
    vector: row-mins -> 16 DVE 32x32 stream-transposes (arranged so each
    block's min-vector lands on partition 0) -> 4 Pool partition_broadcasts
    -> MINV [128,512] -> R = MINV - E (DVE/Pool) -> attT = exp(R) on ACT
  - Z (softmax row sums) comes free from the second matmul: column 0 of each
    v-block holds 1/gamma, so po[:,0] = Z/gamma and the epilogue scale is a
    single reciprocal
  - epilogue scalar_tensor_tensor moved to the (otherwise idle) Pool engine
  - second matmul unchanged: fp32r x fp32r at 1 cyc/row, bf16 output
"""

import sys

sys.path.insert(0, "/opt/trn_rl_repo")

from contextlib import ExitStack

import numpy as np

import concourse.bacc as bacc
import concourse.bass as bass
import concourse.mybir as mybir
import concourse.tile as tile
from concourse import masks
from concourse.bass_utils import run_bass_kernel_spmd

B, C, H, W = 32, 512, 48, 48
N = H * W  # 2304
NP = N + 2  # two 1/gamma cols (even fp32r free sizes) + data
NCORES = 8
SPC = B // NCORES  # samples per core
P = 128
CB = C // P  # 4 channel blocks
KB = N // P  # 18 spatial chunks of 128
NCH2 = [258, 512, 512, 512, 512]  # 2nd-matmul rhs chunking (chunk0 has Z cols)

FP32 = mybir.dt.float32
FP32R = mybir.dt.float32r
FP16 = mybir.dt.float16
BF16 = mybir.dt.bfloat16
AX = mybir.AxisListType.X
OP = mybir.AluOpType
AF = mybir.ActivationFunctionType

LO = [0, P, 2 * P, 3 * P]  # triangular row starts
MIRRORS = {1: [0], 2: [0, 1], 3: [0, 1, 2]}


def _emit(tc, ctx, x, gamma, out, reps=1):
    nc = tc.nc

    const_pool = ctx.enter_context(tc.tile_pool(name="const", bufs=1))
    ident_f32 = const_pool.tile([P, P], FP32)
    masks.make_identity(nc, ident_f32[:])
    ident = const_pool.tile([P, P], FP32R)
    nc.scalar.copy(ident[:], ident_f32[:])
    gamma_sb = const_pool.tile([P, 1], FP32)
    # gamma loads via the ACT DGE queue so the SP queue's head stays free
    # for sample 0's v ranges
    nc.scalar.dma_start(gamma_sb[:], bass.AP(gamma.tensor, 0, [[0, P], [1, 1]]))
    ginv = const_pool.tile([P, 1], FP32)
    nc.vector.reciprocal(ginv[:], gamma_sb[:])

    v_pool = ctx.enter_context(tc.tile_pool(name="v", bufs=2))
    vt_pool = ctx.enter_context(tc.tile_pool(name="vt", bufs=2))
    at_pool = ctx.enter_context(tc.tile_pool(name="at", bufs=2))
    m_pool = ctx.enter_context(tc.tile_pool(name="m", bufs=1))
    o_pool = ctx.enter_context(tc.tile_pool(name="o", bufs=3))
    sc_pool = ctx.enter_context(tc.tile_pool(name="sc", bufs=2))
    r_pool = ctx.enter_context(tc.tile_pool(name="r", bufs=2))
    s_pool = ctx.enter_context(tc.tile_pool(name="s", bufs=2))
    # PSUM: 4 energy banks + 2 rotating transpose banks + 2 output banks
    ps_e = ctx.enter_context(tc.tile_pool(name="ps_e", bufs=1, space="PSUM"))
    ps_t = ctx.enter_context(tc.tile_pool(name="ps_t", bufs=2, space="PSUM"))
    ps_o = ctx.enter_context(tc.tile_pool(name="ps_o", bufs=1, space="PSUM"))

    nsamp = reps * SPC
    v_t = {}
    vt_t = {}

    def load_v(i):
        # one 3-dim DMA per column range delivers that range for ALL 4
        # channel blocks (staggered availability for the just-in-time
        # transposes at minimal instruction-issue cost)
        s = i % SPC
        v = v_pool.tile([P, CB * NP], FP32R, tag="v", name=f"v{i}")
        for cb in range(CB):
            nc.vector.tensor_copy(v[:, cb * NP : cb * NP + 1], ginv[:])
            nc.vector.tensor_copy(v[:, cb * NP + 1 : cb * NP + 2], ginv[:])
        vap = v[:]
        ranges = (
            ((0, 128), (128, 256), (256, 768), (768, 1536), (1536, N))
            if i == 0
            else ((0, 256), (256, 768), (768, 1536), (1536, N))
        )
        for a, b in ranges:
            nc.sync.dma_start(
                bass.AP(
                    vap.tensor, 2 + a, [[CB * NP, P], [NP, CB], [1, b - a]]
                ),
                bass.AP(
                    x.tensor,
                    s * C * N + a,
                    [[N, P], [P * N, CB], [1, b - a]],
                ).bitcast(FP32R),
            )
        v_t[i] = v

    def a_chunk(i, k, copy_eng=None, bank=None):
        # transpose one 128-wide spatial chunk of v into vt16 (fp16 via the
        # PSUM->SBUF copy conversion; transpose data stays fp32r)
        if k == 0:
            vt_t[i] = vt_pool.tile([P, KB * C], FP16, tag="vt", name=f"vt{i}")
        v, vt = v_t[i], vt_t[i]
        if bank is None:
            tps = ps_t.tile([P, 512], FP32R, tag="tps")
        elif bank < CB:
            # freed energy bank (its sample's export_e already drained it;
            # the next sample's energy is emitted after phase D, long after
            # this chunk's copy)
            tps = ps_e.tile([P, 512], FP32R, tag=f"eb{bank}", name=f"tq{i}_{k}")
        else:
            tps = ps_t.tile([P, 512], FP32R, tag="tps")
        for cb in range(CB):
            nc.tensor.matmul(
                tps[:, cb * P : (cb + 1) * P],
                v[:, cb * NP + 2 + k * P : cb * NP + 2 + (k + 1) * P],
                ident[:],
                is_transpose=True,
                start=(cb == 0),
                stop=(cb == CB - 1),
            )
        if copy_eng == "dve":
            nc.vector.tensor_copy(vt[:, k * C : (k + 1) * C], tps[:])
        elif copy_eng == "act":
            nc.scalar.copy(vt[:, k * C : (k + 1) * C], tps[:])
        else:
            nc.any.tensor_copy(vt[:, k * C : (k + 1) * C], tps[:])

    def emit(i):
        s = i % SPC
        v, vt = v_t[i], vt_t.get(i)
        if i + 1 < nsamp:
            load_v(i + 1)
        # one tile per PSUM bank so a block's start-group only guards its bank
        energy = [
            ps_e.tile([P, 512], FP32, tag=f"eb{b}", name=f"energy{i}_{b}")
            for b in range(CB)
        ]
        mns = [
            sc_pool.tile([P, 32], FP32, tag=f"mn{b}", name=f"mn{i}_{b}")
            for b in range(CB)
        ]
        for b in range(CB):
            nc.gpsimd.memset(mns[b][:, 1:32], 0.0)
        Ts = [
            sc_pool.tile([32, P], FP32, tag=f"T{b}", name=f"T{i}_{b}")
            for b in range(CB)
        ]
        p_sb = (
            at_pool.tile([P, CB * 512], FP32R, tag="p", name=f"p{i}")
            if i == nsamp - 1
            else None
        )

        # full E blocks copied PSUM->SBUF right after completion: frees the
        # PSUM bank early (the next sample's energy start-group would
        # otherwise wait for this sample's R subtracts), and the fp32r copy
        # doubles as the mirror source for later row blocks
        e_sb = [None] * CB

        last = i == nsamp - 1

        def minblock(ib):
            # row-mins of E block ib (straight from PSUM, in parallel with
            # the SBUF export) + stream-transposes putting the min vector
            # for channels of block ib on partition 0 of Ts[ib]
            nc.vector.tensor_reduce(
                mns[ib][:, 0:1], energy[ib][:], axis=AX, op=OP.min
            )
            if last:
                # last sample: row-form softmax numerator immediately per
                # block (no MINV wait); the PE attention-transposes below
                # double as chain filler since there is no next sample
                nc.scalar.activation(
                    p_sb[:, ib * 512 : (ib + 1) * 512],
                    energy[ib][:],
                    AF.Exp,
                    bias=mns[ib][:, 0:1],
                    scale=-1.0,
                )
                return
            for j in range(4):
                nc.vector.transpose(
                    Ts[ib][:, 32 * j : 32 * (j + 1)],
                    mns[ib][32 * j : 32 * (j + 1), :],
                )

        # sample 0 uses a flatter triangle: rows 0/1 full-width so its two
        # mirror sources have no serial export->mirror->export chain at the
        # end of the (k-outer) energy loop
        lo = LO
        mirrors = MIRRORS

        def mirrors_for(ib):
            for j, src in enumerate(mirrors.get(ib, ())):
                nc.tensor.matmul(
                    energy[ib][:, src * P : (src + 1) * P].bitcast(FP32R),
                    e_sb[src][:, ib * P : (ib + 1) * P],
                    ident[:],
                    is_transpose=True,
                    start=False,
                    stop=(j == len(mirrors[ib]) - 1),
                )

        def export_e(ib):
            e = m_pool.tile([P, 512], FP32R, tag=f"e{ib}", name=f"esb{i}_{ib}")
            if ib % 2 == 0:
                nc.vector.tensor_copy(e[:], energy[ib][:].bitcast(FP32R))
            else:
                nc.scalar.copy(e[:], energy[ib][:].bitcast(FP32R))
            e_sb[ib] = e

        if i == 0:
            # ---- sample 0: triangular, k-outer, JIT transposes with a
            # 2-chunk lookahead so the PE->copy->PE roundtrip of chunk k
            # overlaps the energy matmuls of chunks k-2/k-1
            a_chunk(0, 0)
            a_chunk(0, 1)
            for k in range(KB):
                vt = vt_t[0]
                for ib in range(CB):
                    nc.tensor.matmul(
                        energy[ib][:, lo[ib] : 512],
                        vt[:, k * C + ib * P : k * C + (ib + 1) * P],
                        vt[:, k * C + lo[ib] : (k + 1) * C],
                        start=(k == 0),
                        stop=(k == KB - 1 and not mirrors.get(ib)),
                    )
                if k + 2 < KB:
                    a_chunk(0, k + 2)
            for ib in range(CB):
                if ib:
                    mirrors_for(ib)
                export_e(ib)
                minblock(ib)
        else:
            # ---- steady: triangular ib-outer; block ib's mirror/export/min
            # chain overlaps block ib+1's k-loop
            for ib in range(CB):
                for k in range(KB):
                    nc.tensor.matmul(
                        energy[ib][:, lo[ib] : 512],
                        vt[:, k * C + ib * P : k * C + (ib + 1) * P],
                        vt[:, k * C + lo[ib] : (k + 1) * C],
                        start=(k == 0),
                        stop=(k == KB - 1 and not mirrors.get(ib)),
                    )
                if ib:
                    mirrors_for(ib)
                export_e(ib)
                minblock(ib)

        att = at_pool.tile([P, CB * 512], FP32R, tag="at", name=f"at{i}")
        if last:
            # ---- last sample: transpose the row-form numerators on PE
            # (fills the tail; all 4 energy banks are free, no next sample)
            pt_ps = [
                ps_e.tile([P, 512], FP32R, tag=f"eb{b}", name=f"ptps{i}_{b}")
                for b in range(CB)
            ]
            for cb in range(CB):
                for db in range(CB):
                    nc.tensor.matmul(
                        pt_ps[db][:, cb * P : (cb + 1) * P],
                        p_sb[:, cb * 512 + db * P : cb * 512 + (db + 1) * P],
                        ident[:],
                        is_transpose=True,
                        start=(cb == 0),
                        stop=(cb == CB - 1),
                    )
            for db in range(CB):
                if db % 2 == 0:
                    nc.vector.tensor_copy(
                        att[:, db * 512 : (db + 1) * 512], pt_ps[db][:]
                    )
                else:
                    nc.scalar.copy(
                        att[:, db * 512 : (db + 1) * 512], pt_ps[db][:]
                    )
        else:
            # ---- MINV broadcast, R = MINV - E, attT = exp(R) ----
            minv = sc_pool.tile([P, 512], FP32, tag="minv", name=f"minv{i}")
            for ib in range(CB):
                nc.gpsimd.partition_broadcast(
                    minv[:, ib * P : (ib + 1) * P], Ts[ib][0:1, :]
                )
            r_t = []
            for db in range(CB):
                r = r_pool.tile([P, 512], FP32, tag=f"r{db}", name=f"r{i}_{db}")
                eng = nc.gpsimd if db > 0 else nc.vector
                eng.tensor_tensor(
                    r[:], minv[:], e_sb[db][:].bitcast(FP32), op=OP.subtract
                )
                r_t.append(r)
            # exps in 256-wide halves, round-robin over db: the first halves
            # unblock ALL of cb0/cb1's phase-D matmuls ~2x sooner than four
            # serial full-width exps would
            for h in range(2):
                for db in range(CB):
                    nc.scalar.activation(
                        att[:, db * 512 + h * 256 : db * 512 + h * 256 + 256],
                        r_t[db][:, h * 256 : h * 256 + 256],
                        AF.Exp,
                    )

        # ALL of the next sample's transposes fill the softmax-chain tail:
        # ~3.8us of PE filler covering the mn->ST->bcast->R->exp chain.
        # They rotate over 6 PSUM banks (4 freed energy + 2 tps) so the
        # PSUM->SBUF copies never block the PE stream; the copies drain on
        # DVE/ACT into phase D, which is then a pure po-matmul stream.
        # bank rotation avoids eb3 (its export sits on the critical chain)
        if i + 1 < nsamp:
            for k in range(KB):
                a_chunk(i + 1, k, copy_eng="act", bank=(k % 5, None)[k % 5 > 2])

        # ---- out = (attT^T @ v) * (gamma/Z) + x; col 0 of each chunk-0
        # matmul is Z/gamma (the 1/gamma column), giving the scale directly
        s_all = s_pool.tile([P, CB], FP32, tag="s", name=f"s{i}")
        chunk_idx = 0

        def po_tile():
            nonlocal chunk_idx
            slot = chunk_idx % 2
            chunk_idx += 1
            return ps_o.tile(
                [P, 512], FP32, tag=f"po{slot}", name=f"po{i}_{chunk_idx}"
            )

        for cb in range(CB):
            # one output tile + one store per channel block: DMA issue time
            # (~1.2us SP seq+HWDGE hold each) dominates small stores
            ot = o_pool.tile([P, N], BF16, tag="ot", name=f"ot{i}_{cb}")
            n_off = 0
            for ci, nch in enumerate(NCH2):
                po = po_tile()
                for db in range(CB):
                    nc.tensor.matmul(
                        po[:, :nch],
                        att[:, db * 512 + cb * P : db * 512 + (cb + 1) * P],
                        v[:, db * NP + n_off : db * NP + n_off + nch],
                        start=(db == 0),
                        stop=(db == CB - 1),
                    )
                if ci == 0:
                    nc.vector.reciprocal(s_all[:, cb : cb + 1], po[:, 0:1])
                    d0, dn = 2, nch - 2
                else:
                    d0, dn = 0, nch
                out_off = n_off + d0 - 2
                # epilogue on DVE (GPSIMD cannot access PSUM)
                eng = nc.vector
                eng.scalar_tensor_tensor(
                    ot[:, out_off : out_off + dn],
                    po[:, d0 : d0 + dn],
                    s_all[:, cb : cb + 1],
                    v[:, cb * NP + n_off + d0 : cb * NP + n_off + d0 + dn].bitcast(
                        FP32
                    ),
                    op0=OP.mult,
                    op1=OP.add,
                )
                n_off += nch
                # two stores per cb: first half ships as soon as its chunks
                # are done (keeps the final drain short)
                if ci == 2:
                    nc.sync.dma_start(
                        out[s, cb * P : (cb + 1) * P, 0:1280], ot[:, 0:1280]
                    )
                elif ci == 3:
                    nc.sync.dma_start(
                        out[s, cb * P : (cb + 1) * P, 1280:1792],
                        ot[:, 1280:1792],
                    )
            nc.sync.dma_start(
                out[s, cb * P : (cb + 1) * P, 1792:N], ot[:, 1792:N]
            )
        del v_t[i], vt_t[i]

    load_v(0)
    for i in range(nsamp):
        emit(i)


_nc_cache = {}


def _build(reps=1):
    if reps in _nc_cache:
        return _nc_cache[reps]
    nc = bacc.Bacc("TRN2", target_bir_lowering=False, debug=False)
    x_d = nc.dram_tensor("x", [SPC, C, N], FP32, kind="ExternalInput")
    g_d = nc.dram_tensor("gamma", [1], FP32, kind="ExternalInput")
    o_d = nc.dram_tensor("out", [SPC, C, N], BF16, kind="ExternalOutput")
    with tile.TileContext(nc) as tc, ExitStack() as ctx:
        _emit(tc, ctx, x_d.ap(), g_d.ap(), o_d.ap(), reps=reps)
    nc.compile()
    _nc_cache[reps] = nc
    return nc


def _bench_fn(reps, x, gamma):
    """Build a jitted 8-core executor for the reps-times-repeated kernel with
    device-resident inputs.  Used by test.py for differential timing."""
    import jax
    from jax.experimental.shard_map import shard_map
    from jax.sharding import Mesh, NamedSharding, PartitionSpec

    from concourse import bass2jax

    bass2jax.install_neuronx_cc_hook()
    nc = _build(reps=reps)
    pid = nc.partition_id_tensor.name if nc.partition_id_tensor else None
    in_names, out_names, out_avals, zero_outs = [], [], [], []
    for alloc in nc.m.functions[0].allocations:
        if not isinstance(alloc, mybir.MemoryLocationSet):
            continue
        name = alloc.memorylocations[0].name
        if alloc.kind == "ExternalInput":
            if name != pid:
                in_names.append(name)
        elif alloc.kind == "ExternalOutput":
            out_names.append(name)
            shape = tuple(alloc.tensor_shape)
            dtype = mybir.dt.np(alloc.dtype)
            out_avals.append(jax.core.ShapedArray(shape, dtype))
            zero_outs.append(np.zeros(shape, dtype))
    all_in_names = list(in_names) + list(out_names)
    if pid:
        all_in_names.append(pid)

    def _body(*args):
        operands = list(args)
        if pid:
            operands.append(bass2jax.partition_id_tensor())
        return tuple(
            bass2jax._bass_exec_p.bind(
                *operands,
                out_avals=tuple(out_avals),
                in_names=tuple(all_in_names),
                out_names=tuple(out_names),
                lowering_input_output_aliases=(),
                sim_require_finite=True,
                sim_require_nnan=True,
                nc=nc,
            )
        )

    devices = jax.devices()[:NCORES]
    mesh = Mesh(np.asarray(devices), ("core",))
    specs = (PartitionSpec("core"),) * (len(in_names) + len(out_names))
    fn = jax.jit(
        shard_map(
            _body,
            mesh=mesh,
            in_specs=specs,
            out_specs=(PartitionSpec("core"),) * len(out_names),
            check_rep=False,
        ),
        keep_unused=True,
    )
    sh = NamedSharding(mesh, PartitionSpec("core"))
    ins = {
        "x": np.ascontiguousarray(x, dtype=np.float32).reshape(B, C, N),
        "gamma": np.tile(np.ascontiguousarray(gamma, dtype=np.float32), (NCORES,)),
    }
    args = [jax.device_put(ins[n], sh) for n in in_names]
    args += [
        jax.device_put(np.zeros((NCORES * z.shape[0], *z.shape[1:]), z.dtype), sh)
        for z in zero_outs
    ]
    return fn, args


def kernel(x: np.ndarray, gamma: np.ndarray, **run_kwargs) -> np.ndarray:
    assert x.shape == (B, C, H, W), x.shape
    nc = _build()
    xr = np.ascontiguousarray(x, dtype=np.float32).reshape(B, C, N)
    g = np.ascontiguousarray(gamma, dtype=np.float32)
    in_maps = [
        {"x": xr[g_idx * SPC : (g_idx + 1) * SPC], "gamma": g}
        for g_idx in range(NCORES)
    ]
    res = run_bass_kernel_spmd(nc, in_maps, core_ids=list(range(NCORES)), **run_kwargs)
    outs = [res.results[g_idx]["out"] for g_idx in range(NCORES)]
    full = np.concatenate(outs, axis=0).reshape(B, C, H, W).astype(np.float32)
    if run_kwargs:
        kernel.last_results = res
    return full


# revision 7
# speedup vs baseline: 1.2573x; 1.2573x over previous
"""CAM (channel attention) kernel V2 for Trainium2, data-parallel over batch.

Per sample:
    v = x.reshape(C, N); energy = v @ v.T
    att = softmax(rowmax(energy) - energy, axis=-1)  [= exp(rowmin-E)/rowsum]
    out = gamma * (att @ v) + x

Key structure changes vs V1:
  - all PE transposes use a BF16 identity: the transpose cycle rate is keyed
    on the moving operand (the identity), so bf16 gives 1.0 cyc/row vs 1.5
    for the fp32r identity (numerically exact - identity is 0/1)
  - energy operands are FP16 (vt16 built by the PSUM->SBUF copy conversion):
    fp16 matmuls run at 1 cyc/row at ANY free width, so the triangular row
    blocks shrink to widths [512,384,256,128] (fp32r needs >=256). fp16
    operand rounding (2^-11) adds ~1.5e-2 logit noise - measured end-to-end
    error stays at the baseline 3.4e-3 (bf16 output rounding dominates)
  - the attention-transpose phase is GONE: since E is symmetric, the
    transposed unnormalized attention is attT[d,c] = exp(min[c] - E[d,c]),
    computed directly on the stored energy tiles with a free-axis bias
    vector: row-mins -> 16 DVE 32x32 stream-transposes (arranged so each
    block's min-vector lands on partition 0) -> 4 Pool partition_broadcasts
    -> MINV [128,512] -> R = MINV - E (DVE/Pool) -> attT = exp(R) on ACT
  - Z (softmax row sums) comes free from the second matmul: column 0 of each
    v-block holds 1/gamma, so po[:,0] = Z/gamma and the epilogue scale is a
    single reciprocal
  - epilogue scalar_tensor_tensor moved to the (otherwise idle) Pool engine
  - second matmul unchanged: fp32r x fp32r at 1 cyc/row, bf16 output
"""

import sys

sys.path.insert(0, "/opt/trn_rl_repo")

from contextlib import ExitStack

import numpy as np

import concourse.bacc as bacc
import concourse.bass as bass
import concourse.mybir as mybir
import concourse.tile as tile
from concourse import masks
from concourse.bass_utils import run_bass_kernel_spmd

B, C, H, W = 32, 512, 48, 48
N = H * W  # 2304
NP = N + 2  # two 1/gamma cols (even fp32r free sizes) + data
NCORES = 8
SPC = B // NCORES  # samples per core
P = 128
CB = C // P  # 4 channel blocks
KB = N // P  # 18 spatial chunks of 128
NCH2 = [258, 512, 512, 512, 512]  # 2nd-matmul rhs chunking (chunk0 has Z cols)

FP32 = mybir.dt.float32
FP32R = mybir.dt.float32r
FP16 = mybir.dt.float16
BF16 = mybir.dt.bfloat16
AX = mybir.AxisListType.X
OP = mybir.AluOpType
AF = mybir.ActivationFunctionType

LO = [0, P, 2 * P, 3 * P]  # triangular row starts
MIRRORS = {1: [0], 2: [0, 1], 3: [0, 1, 2]}


def _emit(tc, ctx, x, gamma, out, reps=1):
    nc = tc.nc

    const_pool = ctx.enter_context(tc.tile_pool(name="const", bufs=1))
    ident_f32 = const_pool.tile([P, P], FP32)
    masks.make_identity(nc, ident_f32[:])
    ident = const_pool.tile([P, P], FP32R)
    nc.scalar.copy(ident[:], ident_f32[:])
    gamma_sb = const_pool.tile([P, 1], FP32)
    # gamma loads via the ACT DGE queue so the SP queue's head stays free
    # for sample 0's v ranges
    nc.scalar.dma_start(gamma_sb[:], bass.AP(gamma.tensor, 0, [[0, P], [1, 1]]))
    ginv = const_pool.tile([P, 1], FP32)
    nc.vector.reciprocal(ginv[:], gamma_sb[:])

    v_pool = ctx.enter_context(tc.tile_pool(name="v", bufs=3))
    vt_pool = ctx.enter_context(tc.tile_pool(name="vt", bufs=2))
    at_pool = ctx.enter_context(tc.tile_pool(name="at", bufs=1))
    m_pool = ctx.enter_context(tc.tile_pool(name="m", bufs=1))
    o_pool = ctx.enter_context(tc.tile_pool(name="o", bufs=3))
    sc_pool = ctx.enter_context(tc.tile_pool(name="sc", bufs=2))
    r_pool = ctx.enter_context(tc.tile_pool(name="r", bufs=1))
    s_pool = ctx.enter_context(tc.tile_pool(name="s", bufs=2))
    # PSUM: 4 energy banks + 2 rotating transpose banks + 2 output banks
    ps_e = ctx.enter_context(tc.tile_pool(name="ps_e", bufs=1, space="PSUM"))
    ps_t = ctx.enter_context(tc.tile_pool(name="ps_t", bufs=2, space="PSUM"))
    ps_o = ctx.enter_context(tc.tile_pool(name="ps_o", bufs=1, space="PSUM"))

    nsamp = reps * SPC
    v_t = {}
    vt_t = {}

    def load_v(i):
        # one 3-dim DMA per column range delivers that range for ALL 4
        # channel blocks (staggered availability for the just-in-time
        # transposes at minimal instruction-issue cost)
        s = i % SPC
        v = v_pool.tile([P, CB * NP], FP32R, tag="v", name=f"v{i}")
        for cb in range(CB):
            nc.vector.tensor_copy(v[:, cb * NP : cb * NP + 1], ginv[:])
            nc.vector.tensor_copy(v[:, cb * NP + 1 : cb * NP + 2], ginv[:])
        vap = v[:]
        ranges = (
            ((0, 128), (128, 256), (256, 768), (768, 1536), (1536, N))
            if i == 0
            else ((0, 256), (256, 768), (768, 1536), (1536, N))
        )
        for a, b in ranges:
            nc.sync.dma_start(
                bass.AP(
                    vap.tensor, 2 + a, [[CB * NP, P], [NP, CB], [1, b - a]]
                ),
                bass.AP(
                    x.tensor,
                    s * C * N + a,
                    [[N, P], [P * N, CB], [1, b - a]],
                ).bitcast(FP32R),
            )
        v_t[i] = v

    def a_chunk(i, k, copy_eng=None, bank=None):
        # transpose one 128-wide spatial chunk of v into vt16 (fp16 via the
        # PSUM->SBUF copy conversion; transpose data stays fp32r)
        if k == 0:
            vt_t[i] = vt_pool.tile([P, KB * C], FP16, tag="vt", name=f"vt{i}")
        v, vt = v_t[i], vt_t[i]
        if bank is None:
            tps = ps_t.tile([P, 512], FP32R, tag="tps")
        elif bank < CB:
            # freed energy bank (its sample's export_e already drained it;
            # the next sample's energy is emitted after phase D, long after
            # this chunk's copy)
            tps = ps_e.tile([P, 512], FP32R, tag=f"eb{bank}", name=f"tq{i}_{k}")
        else:
            tps = ps_t.tile([P, 512], FP32R, tag="tps")
        for cb in range(CB):
            nc.tensor.matmul(
                tps[:, cb * P : (cb + 1) * P],
                v[:, cb * NP + 2 + k * P : cb * NP + 2 + (k + 1) * P],
                ident[:],
                is_transpose=True,
                start=(cb == 0),
                stop=(cb == CB - 1),
            )
        if copy_eng == "dve":
            nc.vector.tensor_copy(vt[:, k * C : (k + 1) * C], tps[:])
        elif copy_eng == "act":
            nc.scalar.copy(vt[:, k * C : (k + 1) * C], tps[:])
        else:
            nc.any.tensor_copy(vt[:, k * C : (k + 1) * C], tps[:])

    def emit(i):
        s = i % SPC
        v, vt = v_t[i], vt_t.get(i)
        if i + 1 < nsamp:
            load_v(i + 1)
        # one tile per PSUM bank so a block's start-group only guards its bank
        energy = [
            ps_e.tile([P, 512], FP32, tag=f"eb{b}", name=f"energy{i}_{b}")
            for b in range(CB)
        ]
        mns = [
            sc_pool.tile([P, 32], FP32, tag=f"mn{b}", name=f"mn{i}_{b}")
            for b in range(CB)
        ]
        for b in range(CB):
            nc.gpsimd.memset(mns[b][:, 1:32], 0.0)
        Ts = [
            sc_pool.tile([32, P], FP32, tag=f"T{b}", name=f"T{i}_{b}")
            for b in range(CB)
        ]
        p_sb = (
            at_pool.tile([P, CB * 512], FP32R, tag="p", name=f"p{i}")
            if i == nsamp - 1
            else None
        )

        # full E blocks copied PSUM->SBUF right after completion: frees the
        # PSUM bank early (the next sample's energy start-group would
        # otherwise wait for this sample's R subtracts), and the fp32r copy
        # doubles as the mirror source for later row blocks
        e_sb = [None] * CB

        last = i == nsamp - 1

        def minblock(ib):
            # row-mins of E block ib (straight from PSUM, in parallel with
            # the SBUF export) + stream-transposes putting the min vector
            # for channels of block ib on partition 0 of Ts[ib]
            nc.vector.tensor_reduce(
                mns[ib][:, 0:1], energy[ib][:], axis=AX, op=OP.min
            )
            if last:
                # last sample: row-form softmax numerator immediately per
                # block (no MINV wait); the PE attention-transposes below
                # double as chain filler since there is no next sample
                nc.scalar.activation(
                    p_sb[:, ib * 512 : (ib + 1) * 512],
                    energy[ib][:],
                    AF.Exp,
                    bias=mns[ib][:, 0:1],
                    scale=-1.0,
                )
                return
            for j in range(4):
                nc.vector.transpose(
                    Ts[ib][:, 32 * j : 32 * (j + 1)],
                    mns[ib][32 * j : 32 * (j + 1), :],
                )

        # sample 0 uses a flatter triangle: rows 0/1 full-width so its two
        # mirror sources have no serial export->mirror->export chain at the
        # end of the (k-outer) energy loop
        lo = LO
        mirrors = MIRRORS

        def mirrors_for(ib):
            for j, src in enumerate(mirrors.get(ib, ())):
                nc.tensor.matmul(
                    energy[ib][:, src * P : (src + 1) * P].bitcast(FP32R),
                    e_sb[src][:, ib * P : (ib + 1) * P],
                    ident[:],
                    is_transpose=True,
                    start=False,
                    stop=(j == len(mirrors[ib]) - 1),
                )

        def export_e(ib):
            e = m_pool.tile([P, 512], FP32R, tag=f"e{ib}", name=f"esb{i}_{ib}")
            if ib % 2 == 0:
                nc.vector.tensor_copy(e[:], energy[ib][:].bitcast(FP32R))
            else:
                nc.scalar.copy(e[:], energy[ib][:].bitcast(FP32R))
            e_sb[ib] = e

        if i == 0:
            # ---- sample 0: triangular, k-outer, JIT transposes with a
            # 2-chunk lookahead so the PE->copy->PE roundtrip of chunk k
            # overlaps the energy matmuls of chunks k-2/k-1
            a_chunk(0, 0)
            a_chunk(0, 1)
            for k in range(KB):
                vt = vt_t[0]
                for ib in range(CB):
                    nc.tensor.matmul(
                        energy[ib][:, lo[ib] : 512],
                        vt[:, k * C + ib * P : k * C + (ib + 1) * P],
                        vt[:, k * C + lo[ib] : (k + 1) * C],
                        start=(k == 0),
                        stop=(k == KB - 1 and not mirrors.get(ib)),
                    )
                if k + 2 < KB:
                    a_chunk(0, k + 2)
            for ib in range(CB):
                if ib:
                    mirrors_for(ib)
                export_e(ib)
                minblock(ib)
        else:
            # ---- steady: triangular ib-outer; block ib's mirror/export/min
            # chain overlaps block ib+1's k-loop
            for ib in range(CB):
                for k in range(KB):
                    nc.tensor.matmul(
                        energy[ib][:, lo[ib] : 512],
                        vt[:, k * C + ib * P : k * C + (ib + 1) * P],
                        vt[:, k * C + lo[ib] : (k + 1) * C],
                        start=(k == 0),
                        stop=(k == KB - 1 and not mirrors.get(ib)),
                    )
                if ib:
                    mirrors_for(ib)
                export_e(ib)
                minblock(ib)

        att = at_pool.tile([P, CB * 512], FP32R, tag="at", name=f"at{i}")
        if last:
            # ---- last sample: transpose the row-form numerators on PE
            # (fills the tail; all 4 energy banks are free, no next sample)
            pt_ps = [
                ps_e.tile([P, 512], FP32R, tag=f"eb{b}", name=f"ptps{i}_{b}")
                for b in range(CB)
            ]
            for cb in range(CB):
                for db in range(CB):
                    nc.tensor.matmul(
                        pt_ps[db][:, cb * P : (cb + 1) * P],
                        p_sb[:, cb * 512 + db * P : cb * 512 + (db + 1) * P],
                        ident[:],
                        is_transpose=True,
                        start=(cb == 0),
                        stop=(cb == CB - 1),
                    )
            for db in range(CB):
                if db % 2 == 0:
                    nc.vector.tensor_copy(
                        att[:, db * 512 : (db + 1) * 512], pt_ps[db][:]
                    )
                else:
                    nc.scalar.copy(
                        att[:, db * 512 : (db + 1) * 512], pt_ps[db][:]
                    )
        else:
            # ---- MINV broadcast, R = MINV - E, attT = exp(R) ----
            minv = sc_pool.tile([P, 512], FP32, tag="minv", name=f"minv{i}")
            for ib in range(CB):
                nc.gpsimd.partition_broadcast(
                    minv[:, ib * P : (ib + 1) * P], Ts[ib][0:1, :]
                )
            r_t = []
            for db in range(CB):
                r = r_pool.tile([P, 512], FP32, tag=f"r{db}", name=f"r{i}_{db}")
                eng = nc.gpsimd if db > 0 else nc.vector
                eng.tensor_tensor(
                    r[:], minv[:], e_sb[db][:].bitcast(FP32), op=OP.subtract
                )
                r_t.append(r)
            # exps in 256-wide halves, round-robin over db: the first halves
            # unblock ALL of cb0/cb1's phase-D matmuls ~2x sooner than four
            # serial full-width exps would
            for h in range(2):
                for db in range(CB):
                    nc.scalar.activation(
                        att[:, db * 512 + h * 256 : db * 512 + h * 256 + 256],
                        r_t[db][:, h * 256 : h * 256 + 256],
                        AF.Exp,
                    )

        # ALL of the next sample's transposes fill the softmax-chain tail:
        # ~3.8us of PE filler covering the mn->ST->bcast->R->exp chain.
        # They rotate over 6 PSUM banks (4 freed energy + 2 tps) so the
        # PSUM->SBUF copies never block the PE stream; the copies drain on
        # DVE/ACT into phase D, which is then a pure po-matmul stream.
        # bank rotation avoids eb3 (its export sits on the critical chain)
        if i + 1 < nsamp:
            for k in range(KB):
                a_chunk(i + 1, k, copy_eng="act", bank=(k % 5, None)[k % 5 > 2])

        # ---- out = (attT^T @ v) * (gamma/Z) + x; col 0 of each chunk-0
        # matmul is Z/gamma (the 1/gamma column), giving the scale directly
        s_all = s_pool.tile([P, CB], FP32, tag="s", name=f"s{i}")
        chunk_idx = 0

        def po_tile():
            nonlocal chunk_idx
            slot = chunk_idx % 2
            chunk_idx += 1
            return ps_o.tile(
                [P, 512], FP32, tag=f"po{slot}", name=f"po{i}_{chunk_idx}"
            )

        for cb in range(CB):
            # one output tile + one store per channel block: DMA issue time
            # (~1.2us SP seq+HWDGE hold each) dominates small stores
            ot = o_pool.tile([P, N], BF16, tag="ot", name=f"ot{i}_{cb}")
            n_off = 0
            for ci, nch in enumerate(NCH2):
                po = po_tile()
                for db in range(CB):
                    nc.tensor.matmul(
                        po[:, :nch],
                        att[:, db * 512 + cb * P : db * 512 + (cb + 1) * P],
                        v[:, db * NP + n_off : db * NP + n_off + nch],
                        start=(db == 0),
                        stop=(db == CB - 1),
                    )
                if ci == 0:
                    nc.vector.reciprocal(s_all[:, cb : cb + 1], po[:, 0:1])
                    d0, dn = 2, nch - 2
                else:
                    d0, dn = 0, nch
                out_off = n_off + d0 - 2
                # epilogue on DVE (GPSIMD cannot access PSUM)
                eng = nc.vector
                eng.scalar_tensor_tensor(
                    ot[:, out_off : out_off + dn],
                    po[:, d0 : d0 + dn],
                    s_all[:, cb : cb + 1],
                    v[:, cb * NP + n_off + d0 : cb * NP + n_off + d0 + dn].bitcast(
                        FP32
                    ),
                    op0=OP.mult,
                    op1=OP.add,
                )
                n_off += nch
                # two stores per cb: first half ships as soon as its chunks
                # are done (keeps the final drain short)
                if ci == 2:
                    nc.sync.dma_start(
                        out[s, cb * P : (cb + 1) * P, 0:1280], ot[:, 0:1280]
                    )
                elif ci == 3:
                    nc.sync.dma_start(
                        out[s, cb * P : (cb + 1) * P, 1280:1792],
                        ot[:, 1280:1792],
                    )
            nc.sync.dma_start(
                out[s, cb * P : (cb + 1) * P, 1792:N], ot[:, 1792:N]
            )
        del v_t[i], vt_t[i]

    load_v(0)
    for i in range(nsamp):
        emit(i)


_nc_cache = {}


def _build(reps=1):
    if reps in _nc_cache:
        return _nc_cache[reps]
    nc = bacc.Bacc("TRN2", target_bir_lowering=False, debug=False)
    x_d = nc.dram_tensor("x", [SPC, C, N], FP32, kind="ExternalInput")
    g_d = nc.dram_tensor("gamma", [1], FP32, kind="ExternalInput")
    o_d = nc.dram_tensor("out", [SPC, C, N], BF16, kind="ExternalOutput")
    with tile.TileContext(nc) as tc, ExitStack() as ctx:
        _emit(tc, ctx, x_d.ap(), g_d.ap(), o_d.ap(), reps=reps)
    nc.compile()
    _nc_cache[reps] = nc
    return nc


def _bench_fn(reps, x, gamma):
    """Build a jitted 8-core executor for the reps-times-repeated kernel with
    device-resident inputs.  Used by test.py for differential timing."""
    import jax
    from jax.experimental.shard_map import shard_map
    from jax.sharding import Mesh, NamedSharding, PartitionSpec

    from concourse import bass2jax

    bass2jax.install_neuronx_cc_hook()
    nc = _build(reps=reps)
    pid = nc.partition_id_tensor.name if nc.partition_id_tensor else None
    in_names, out_names, out_avals, zero_outs = [], [], [], []
    for alloc in nc.m.functions[0].allocations:
        if not isinstance(alloc, mybir.MemoryLocationSet):
            continue
        name = alloc.memorylocations[0].name
        if alloc.kind == "ExternalInput":
            if name != pid:
                in_names.append(name)
        elif alloc.kind == "ExternalOutput":
            out_names.append(name)
            shape = tuple(alloc.tensor_shape)
            dtype = mybir.dt.np(alloc.dtype)
            out_avals.append(jax.core.ShapedArray(shape, dtype))
            zero_outs.append(np.zeros(shape, dtype))
    all_in_names = list(in_names) + list(out_names)
    if pid:
        all_in_names.append(pid)

    def _body(*args):
        operands = list(args)
        if pid:
            operands.append(bass2jax.partition_id_tensor())
        return tuple(
            bass2jax._bass_exec_p.bind(
                *operands,
                out_avals=tuple(out_avals),
                in_names=tuple(all_in_names),
                out_names=tuple(out_names),
                lowering_input_output_aliases=(),
                sim_require_finite=True,
                sim_require_nnan=True,
                nc=nc,
            )
        )

    devices = jax.devices()[:NCORES]
    mesh = Mesh(np.asarray(devices), ("core",))
    specs = (PartitionSpec("core"),) * (len(in_names) + len(out_names))
    fn = jax.jit(
        shard_map(
            _body,
            mesh=mesh,
            in_specs=specs,
            out_specs=(PartitionSpec("core"),) * len(out_names),
            check_rep=False,
        ),
        keep_unused=True,
    )
    sh = NamedSharding(mesh, PartitionSpec("core"))
    ins = {
        "x": np.ascontiguousarray(x, dtype=np.float32).reshape(B, C, N),
        "gamma": np.tile(np.ascontiguousarray(gamma, dtype=np.float32), (NCORES,)),
    }
    args = [jax.device_put(ins[n], sh) for n in in_names]
    args += [
        jax.device_put(np.zeros((NCORES * z.shape[0], *z.shape[1:]), z.dtype), sh)
        for z in zero_outs
    ]
    return fn, args


def kernel(x: np.ndarray, gamma: np.ndarray, **run_kwargs) -> np.ndarray:
    assert x.shape == (B, C, H, W), x.shape
    nc = _build()
    xr = np.ascontiguousarray(x, dtype=np.float32).reshape(B, C, N)
    g = np.ascontiguousarray(gamma, dtype=np.float32)
    in_maps = [
        {"x": xr[g_idx * SPC : (g_idx + 1) * SPC], "gamma": g}
        for g_idx in range(NCORES)
    ]
    res = run_bass_kernel_spmd(nc, in_maps, core_ids=list(range(NCORES)), **run_kwargs)
    outs = [res.results[g_idx]["out"] for g_idx in range(NCORES)]
    full = np.concatenate(outs, axis=0).reshape(B, C, H, W).astype(np.float32)
    if run_kwargs:
        kernel.last_results = res
    return full
